# revision 38
# baseline (speedup 1.0000x reference)
"""Multi-head causal attention (B=2, S=2048, D=1024, H=16, Dh=64) on 8
axon-tunneled TRN2 NeuronCores.

Sharding: core = b*4 + g handles batch b and head group g (4 heads, 256
feature columns of the QKV projections / 256 rows of Wo).  Each core is
fully independent; the host sums the 4 per-head-group partial outputs of
each batch.

Per-core layout ("feature on partitions, seq on free"):
  xT   (1024, 2048)  = x[b].T                       (bf16)
  QT   (256, 2048)   = (0.125*Wq_g).T @ x.T + 0.125*bq_g   (scale in Wq)
  KT   (256, 2048)   = Wk_g.T @ x.T + bk_g
  va   (2048, 386)   = per even head [V_h | 1]; per odd head
                       [0*32 | 1 | 0*31 | V_h]  (so the PV matmul puts the
                       odd head's ctx at partitions 64:128 and its softmax
                       denominator at partition 32 - no partition-shifting
                       DMA needed afterwards)
  S^T tiles (128k, 512q) = KT_h[:, kblk].T @ QT_h[:, qchunk]  (contract 64)
  P^T  = exp(S^T) * mask01          (binary causal mask applied post-exp;
                                     no max-subtraction: |S| small)
  ctx_aug^T = sum_k va_h[kblk].T @ P^T              (PSUM accumulate)
  ctxT (256, 2048) = ctx_aug * (1/denom) + bv_h
  out_partial (2048, 1024) = ctxT.T @ Wo_g          (bf16, host sums in fp32)

Attention is software-pipelined: the two heads of a pair issue their
score matmuls back-to-back into different PE row groups (they run
concurrently - contraction is only 64), and the PV matmuls lag the score
matmuls by one k-block so the PE never sits behind the exp on the
scalar engine.  Normalization + output projection of a finished group
are emitted one group later, hiding the reciprocal-chain latency.
"""

import numpy as np

D_IN = 1024
D_OUT = 1024
H = 16
DH = 64
B = 2
S = 2048
NCORES = 8
HG = 4            # heads per core
DG = HG * DH      # 256 feature cols per core

MM_DT_NAME = "bfloat16"

_state = {}


def _patch_tile_drain():
    """This image's walrus rejects instructions carrying >2 sync waits
    ("Too many sync wait commands"); Tile's final drain waits on every
    outstanding proc.  Split the waits into single-wait SP nops."""
    import concourse.tile as tile
    from concourse import mybir
    from concourse.vector_clock import ScopedClock

    if getattr(tile.TileContext._drain_and_barrier, "_split_waits", False):
        return

    def _drain_and_barrier(self, tick_clock, wait_clock):
        nc = self.nc
        probe = nc.sync.nop()
        wait_clock.add_sem_waits(
            probe.ins, ScopedClock({None: tick_clock.global_clock})
        )
        si = probe.ins.sync_info
        waits = list(si.on_wait) if si and si.on_wait else []
        if len(waits) > 1:
            probe.ins.sync_info = mybir.SyncInfo(
                on_wait=[waits[0]], on_update=list(si.on_update or [])
            )
            for w in waits[1:]:
                extra = nc.sync.nop()
                extra.ins.sync_info = mybir.SyncInfo(on_wait=[w], on_update=[])
        nc.sync.drain()

        nc.all_engine_barrier()
        assert self.sems is not None
        popped = nc._tile_sem_poison_stack.pop()
        assert popped is self._sem_poison
        nc.clear_and_free_semaphores(list(self.sems.allocated().values()))
        nc.all_engine_barrier()

    _drain_and_barrier._split_waits = True
    tile.TileContext._drain_and_barrier = _drain_and_barrier


def _split_excess_waits(nc, maxw=1):
    """Walrus in this image rejects instructions with too many sync-wait
    commands.  Hoist excess waits onto InstNoOp carriers inserted right
    before the offending instruction on the same engine (engines are
    in-order, so this preserves semantics)."""
    from concourse import mybir

    f = nc.m.functions[0]
    for bb in f.blocks:
        insts = bb.instructions  # live list
        i = 0
        while i < len(insts):
            ins = insts[i]
            si = ins.sync_info
            waits = list(si.on_wait) if si and si.on_wait else []
            if len(waits) > maxw:
                excess, keep = waits[:-maxw], waits[-maxw:]
                nops = []
                for j in range(0, len(excess), maxw):
                    nop = mybir.InstNoOp(
                        name=f"I-waitnop-{nc.next_id()}", ins=[], outs=[]
                    )
                    nop.engine = ins.engine
                    nop.sync_info = mybir.SyncInfo(
                        on_wait=excess[j : j + maxw], on_update=[]
                    )
                    nops.append(nop)
                ins.sync_info = mybir.SyncInfo(
                    on_wait=keep, on_update=list(si.on_update or [])
                )
                insts[i:i] = nops
                i += len(nops)
            i += 1


# va column layout: even heads [V|1] (65 cols), odd heads
# [1 | zeros*63 | V] (128 cols - the ones column at position 0 puts the
# odd head's softmax denominator at out partition 0, its ctx at 64:128).
# Offsets per head:
VA_OFF = [0, 65, 193, 258]
VA_COLS = 386


def _build_nc():
    import concourse.bass as bass
    import concourse.tile as tile
    from concourse import mybir

    _patch_tile_drain()
    FP = mybir.dt.float32
    R = mybir.dt.float32r
    Alu = mybir.AluOpType
    Act = mybir.ActivationFunctionType

    assert MM_DT_NAME == "bfloat16"
    MD = mybir.dt.bfloat16

    nc = bass.Bass("TRN2", target_bir_lowering=False, debug=False)
    d_xT = nc.dram_tensor("xT", [8, 128, S], MD, kind="ExternalInput").ap()
    d_wq = nc.dram_tensor("wq", [8, 128, DG], MD, kind="ExternalInput").ap()
    d_wk = nc.dram_tensor("wk", [8, 128, DG], MD, kind="ExternalInput").ap()
    d_wv = nc.dram_tensor("wv", [8, 128, DG], MD, kind="ExternalInput").ap()
    d_wo = nc.dram_tensor("wo", [2, 128, D_OUT], MD, kind="ExternalInput").ap()
    d_bq = nc.dram_tensor("bq", [2, 128, 1], FP, kind="ExternalInput").ap()
    d_bk = nc.dram_tensor("bk", [2, 128, 1], FP, kind="ExternalInput").ap()
    d_bv = nc.dram_tensor("bv", [2, 128, 1], FP, kind="ExternalInput").ap()
    d_mask = nc.dram_tensor("masks", [4, 128, 512], MD, kind="ExternalInput").ap()
    d_out = nc.dram_tensor("out", [S, D_OUT], MD, kind="ExternalOutput").ap()

    with tile.TileContext(nc) as tc:
        from contextlib import ExitStack

        with ExitStack() as ctx:
            const = ctx.enter_context(tc.tile_pool(name="const", bufs=1))
            qkv = ctx.enter_context(tc.tile_pool(name="qkv", bufs=1))

            wq_sb = [const.tile([128, DG], MD, tag=f"wq{i}", name=f"wq{i}") for i in range(8)]
            wk_sb = [const.tile([128, DG], MD, tag=f"wk{i}", name=f"wk{i}") for i in range(8)]
            wv_sb = [const.tile([128, DG], MD, tag=f"wv{i}", name=f"wv{i}") for i in range(8)]
            wo_sb = [const.tile([128, D_OUT], MD, tag=f"wo{i}", name=f"wo{i}") for i in range(2)]
            bq_sb = [const.tile([128, 1], FP, tag=f"bq{i}", name=f"bq{i}") for i in range(2)]
            bk_sb = [const.tile([128, 1], FP, tag=f"bk{i}", name=f"bk{i}") for i in range(2)]
            bv_sb = [const.tile([128, 1], FP, tag=f"bv{i}", name=f"bv{i}") for i in range(2)]
            mask_sb = [const.tile([128, 512], MD, tag=f"mask{j}", name=f"mask{j}") for j in range(4)]
            # broadcast-ones rows for the denominator matmuls: row 64 feeds
            # even heads (denom at partition 64), row 32 odd heads (partition
            # 32).  fp32r: full accuracy, 2-cycles/row matmul (memset cannot
            # target fp32r, so write through a fp32 bitcast view).
            ones_sb = const.tile([65, DH], R, tag="ones")
            nc.vector.memset(ones_sb[64:65, :].bitcast(FP), 1.0)
            # full-width ones row at partition 0 for the odd-head broadcast
            # (dst partitions 64:128 require a 128-col matmul: col-group-64
            # dst with a 32-row tile fails the s3d3 ISA dst-partition check)
            ones2_sb = const.tile([1, 128], R, tag="ones2")
            nc.vector.memset(ones2_sb[:, :].bitcast(FP), 1.0)

            qT = [qkv.tile([128, S], MD, tag=f"qT{i}", name=f"qT{i}") for i in range(2)]
            kT = [qkv.tile([128, S], MD, tag=f"kT{i}", name=f"kT{i}") for i in range(2)]
            va = [qkv.tile([128, VA_COLS], MD, tag=f"va{i}", name=f"va{i}") for i in range(16)]
            ctxT = [qkv.tile([128, S], MD, tag=f"ctxT{i}", name=f"ctxT{i}") for i in range(2)]

            # va ones columns + odd-head zero padding
            for st in range(16):
                for h in (1, 3):
                    off = VA_OFF[h]
                    nc.vector.memset(va[st][:, off : off + 64], 0.0)
                    nc.vector.memset(va[st][:, off : off + 1], 1.0)
                for h in (0, 2):
                    off = VA_OFF[h]
                    nc.vector.memset(va[st][:, off + 64 : off + 65], 1.0)

            # ---------------- phase 1: projections ----------------
            # xpool stays open through phase 2: V-projection groups for
            # s-tiles 4..15 are interleaved between attention groups (the
            # exp-paced attention loop leaves the PE ~40% idle)
            xpool = ctx.enter_context(tc.tile_pool(name="xp", bufs=1))
            if True:
                xsb = [xpool.tile([128, S], MD, tag=f"x{i}", name=f"x{i}") for i in range(8)]
                # PE warm-up: dummy matmuls on resident scratch bridge the
                # initial input-DMA wait so the HAM clock gate opens before
                # the first real matmul (cold PE runs at 1.2 instead of
                # 2.4 GHz for its first ~3.4us of activity)
                warm = const.tile([1, 512], MD, tag="warm")
                nc.vector.memset(warm[:, :], 1.0)
                # spread input DMA over the three DMA-capable queues so the
                # x tiles land as fast as the fabric allows
                qs3 = [nc.sync, nc.scalar, nc.gpsimd]
                for i in range(8):
                    qs3[i % 3].dma_start(xsb[i][:], d_xT[i])
                    qs3[(i + 1) % 3].dma_start(wq_sb[i][:], d_wq[i])
                    qs3[(i + 2) % 3].dma_start(wk_sb[i][:], d_wk[i])
                    qs3[(i + 1) % 3].dma_start(wv_sb[i][:], d_wv[i])
                for i in range(2):
                    nc.scalar.dma_start(bq_sb[i][:], d_bq[i])
                    nc.gpsimd.dma_start(bk_sb[i][:], d_bk[i])
                    nc.sync.dma_start(bv_sb[i][:], d_bv[i])
                    nc.gpsimd.dma_start(wo_sb[i][:], d_wo[i])
                for j in range(4):
                    nc.scalar.dma_start(mask_sb[j][:], d_mask[j])

                # Q/K projections, ci-outer so the accumulation matmuls
                # pipeline with the x-tile DMA arrival order
                with tc.tile_pool(name="qkp", bufs=1, space="PSUM") as qkp:
                    wps = qkp.tile([128, 512], FP, tag="pq0", name="warmps")
                    for r in range(24):
                        nc.tensor.matmul(
                            wps[:, :], warm[0:1, 0:128], warm[0:1, :],
                            start=True, stop=True,
                        )
                    for m in range(2):
                        ms = slice(m * 128, (m + 1) * 128)
                        psQ = [qkp.tile([128, 512], FP, tag=f"pq{nq}", name=f"pq{m}{nq}") for nq in range(4)]
                        psK = [qkp.tile([128, 512], FP, tag=f"pk{nq}", name=f"pk{m}{nq}") for nq in range(4)]
                        for ci in range(8):
                            for nq in range(4):
                                sq = slice(nq * 512, (nq + 1) * 512)
                                nc.tensor.matmul(
                                    psQ[nq][:], wq_sb[ci][:, ms], xsb[ci][:, sq],
                                    start=(ci == 0), stop=(ci == 7),
                                )
                            for nq in range(4):
                                sq = slice(nq * 512, (nq + 1) * 512)
                                nc.tensor.matmul(
                                    psK[nq][:], wk_sb[ci][:, ms], xsb[ci][:, sq],
                                    start=(ci == 0), stop=(ci == 7),
                                )
                        for nq in range(4):
                            sq = slice(nq * 512, (nq + 1) * 512)
                            nc.vector.tensor_scalar(
                                qT[m][:, sq], psQ[nq][:], bq_sb[m][:], None, Alu.add
                            )
                            nc.vector.tensor_scalar(
                                kT[m][:, sq], psK[nq][:], bk_sb[m][:], None, Alu.add
                            )

            # ------- phase 2+3: pipelined attention + output projection -----
            # PSUM budget (8 banks): sps ring 2 + cps 2 tags x 2 bufs = 4 +
            # outproj/broadcast shared ring 2.  cps double-buffering is what
            # lets group g+1's first PV matmuls run while group g's
            # normalization chain (Ln/Exp on ACT) is still in flight.
            with tc.tile_pool(name="pt", bufs=4) as ptp, tc.tile_pool(
                name="norm", bufs=2
            ) as normp, tc.tile_pool(name="osb", bufs=3) as osb, tc.tile_pool(
                name="spsum", bufs=2, space="PSUM"
            ) as sp, tc.tile_pool(
                name="cpsum", bufs=1, space="PSUM"
            ) as cp, tc.tile_pool(
                name="opsum", bufs=2, space="PSUM"
            ) as op:

                def emit_vgroup(st):
                    """V-projection for s-tile st -> va[st] (psum from the
                    shared outproj ring; PE work fills exp-paced idle)."""
                    ps = op.tile([128, DG], FP, tag="o", name=f"pv{st}")
                    ss = slice(st * 128, (st + 1) * 128)
                    for ci in range(8):
                        nc.tensor.matmul(
                            ps[:], xsb[ci][:, ss], wv_sb[ci][:],
                            start=(ci == 0), stop=(ci == 7),
                        )
                    for h in range(HG):
                        dst0 = VA_OFF[h] + (0 if h % 2 == 0 else 64)
                        nc.vector.tensor_copy(
                            va[st][:, dst0 : dst0 + 64],
                            ps[:, h * 64 : (h + 1) * 64],
                        )

                def emit_norm(qc, ht, cps0, cps1):
                    """Normalize both heads of pair (qc, ht) and write ctxT."""
                    qsl = slice(qc * 512, (qc + 1) * 512)
                    rec0 = normp.tile([65, 512], FP, tag="rec0", name="rec0")
                    rec = normp.tile([65, 512], R, tag="rec", name="rec")
                    # gather both denominators (even head: partition 64 of
                    # cps0, odd head: partition 0 of cps1) into one tile so a
                    # single Ln+Exp pass covers the pair (rows 1..63 are
                    # garbage - processed but never read)
                    dd = normp.tile([65, 512], FP, tag="dd", name="dd")
                    nc.vector.tensor_copy(dd[64:65, :], cps0[64:65, :])
                    nc.vector.tensor_copy(dd[0:1, :], cps1[0:1, :])
                    nc.scalar.activation(rec0[0:65, :], dd[0:65, :], Act.Ln)
                    nc.scalar.activation(
                        rec[0:65, :], rec0[0:65, :], Act.Exp, scale=-1.0
                    )
                    # odd-head broadcast first (writes all 128 partitions;
                    # only 64:128 are consumed), then the even-head 64-row
                    # broadcast overwrites partitions 0:64
                    bc = op.tile([128, 512], FP, tag="o", name="bc")
                    nc.tensor.matmul(
                        bc[:, :], ones2_sb[0:1, :], rec[0:1, :],
                        start=True, stop=True,
                    )
                    bcs = normp.tile([128, 512], FP, tag="bcs", name="bcs")
                    nc.vector.tensor_copy(bcs[64:128, :], bc[64:128, :])
                    nc.tensor.matmul(
                        bc[0:64, :], ones_sb[64:65, :], rec[64:65, :],
                        start=True, stop=True,
                    )
                    nc.vector.tensor_copy(bcs[0:64, :], bc[0:64, :])
                    tmp = normp.tile([128, 512], FP, tag="tmp", name="tmp")
                    nc.vector.tensor_mul(tmp[0:64, :], cps0[0:64, :], bcs[0:64, :])
                    nc.vector.tensor_mul(
                        tmp[64:128, :], cps1[64:128, :], bcs[64:128, :]
                    )
                    # bias add on DVE, both heads in one op (gpsimd tensor
                    # ops cost ~7.5us each - keep it off gpsimd)
                    nc.vector.tensor_scalar(
                        ctxT[ht][:, qsl], tmp[:, :], bv_sb[ht][:, :],
                        None, Alu.add,
                    )

                def emit_outproj(qc):
                    oq = [nc.gpsimd, nc.sync, nc.scalar]
                    for st in range(4 * qc, 4 * qc + 4):
                        ss = slice(st * 128, (st + 1) * 128)
                        ot = osb.tile([128, 1024], MD, tag="ot", name="ot")
                        for n in range(2):
                            ns = slice(n * 512, (n + 1) * 512)
                            ps = op.tile([128, 512], FP, tag="o", name="o")
                            for cb in range(2):
                                nc.tensor.matmul(
                                    ps[:], ctxT[cb][:, ss], wo_sb[cb][:, ns],
                                    start=(cb == 0), stop=(cb == 1),
                                )
                            nc.vector.tensor_copy(ot[:, ns], ps[:])
                        oq[st % 3].dma_start(d_out[ss, :], ot[:])

                for st in range(4):
                    emit_vgroup(st)

                pending = None      # deferred norm emitter (fires at kb==1)
                pending_out = None  # deferred outproj emitter (fires at kb==4)
                for qc in range(4):
                    qs0 = qc * 512
                    for ht in range(2):
                        nkb = 4 * qc + 4
                        cps0 = cp.tile([65, 512], FP, tag="c0", name="c0")
                        cps1 = cp.tile([128, 512], FP, tag="c1", name="c1")
                        prev = None  # (kb, lo, pt0, pt1)
                        for kb in range(nkb):
                            ks = slice(kb * 128, (kb + 1) * 128)
                            j = kb - 4 * qc
                            lo = 128 * j if j > 0 else 0
                            qsl = slice(qs0 + lo, qs0 + 512)
                            # score matmuls for both heads, back-to-back into
                            # the two banks of one [128,1024] PSUM tile:
                            # different PE row groups -> run concurrently
                            sps = sp.tile([128, 1024], FP, tag="s", name="s")
                            for hp in range(2):
                                hs = slice(hp * 64, hp * 64 + 64)
                                nc.tensor.matmul(
                                    sps[:, 512 * hp + lo : 512 * (hp + 1)],
                                    kT[ht][hs, ks], qT[ht][hs, qsl],
                                    start=True, stop=True,
                                )
                            if kb == 1 and pending is not None:
                                pending()
                                pending = None
                            if kb == 4 and pending_out is not None:
                                pending_out()
                                pending_out = None
                            pt = ptp.tile([128, 1024], MD, tag="pt", name="pt")
                            if j < 0:
                                # off-diagonal: both halves fully live -> one
                                # wide exp covers the pair
                                nc.scalar.activation(pt[:, :], sps[:, :], Act.Exp)
                            else:
                                # diagonal: exp each half's written region,
                                # then zero the causal triangle post-exp with
                                # a binary bf16 mask (2x DVE)
                                for hp in range(2):
                                    h0c = 512 * hp + lo
                                    h1c = 512 * (hp + 1)
                                    nc.scalar.activation(
                                        pt[:, h0c:h1c], sps[:, h0c:h1c], Act.Exp
                                    )
                                    nc.vector.tensor_mul(
                                        pt[:, h0c:h1c], pt[:, h0c:h1c],
                                        mask_sb[j][:, lo:],
                                    )
                            if prev is not None:
                                pkb, plo, ppt = prev
                                h0, h1 = 2 * ht, 2 * ht + 1
                                nc.tensor.matmul(
                                    cps0[:, plo:],
                                    va[pkb][:, VA_OFF[h0] : VA_OFF[h0] + 65],
                                    ppt[:, plo:512],
                                    start=(pkb == 0), stop=(pkb == nkb - 1),
                                )
                                nc.tensor.matmul(
                                    cps1[:, plo:],
                                    va[pkb][:, VA_OFF[h1] : VA_OFF[h1] + 128],
                                    ppt[:, 512 + plo : 1024],
                                    start=(pkb == 0), stop=(pkb == nkb - 1),
                                )
                            prev = (kb, lo, pt)
                        # drain the last block's PV pair
                        pkb, plo, ppt = prev
                        h0, h1 = 2 * ht, 2 * ht + 1
                        nc.tensor.matmul(
                            cps0[:, plo:],
                            va[pkb][:, VA_OFF[h0] : VA_OFF[h0] + 65],
                            ppt[:, plo:512],
                            start=(pkb == 0), stop=(pkb == nkb - 1),
                        )
                        nc.tensor.matmul(
                            cps1[:, plo:],
                            va[pkb][:, VA_OFF[h1] : VA_OFF[h1] + 128],
                            ppt[:, 512 + plo : 1024],
                            start=(pkb == 0), stop=(pkb == nkb - 1),
                        )

                        gi = 2 * qc + ht
                        if gi < 6:
                            emit_vgroup(4 + 2 * gi)
                            emit_vgroup(5 + 2 * gi)

                        def make_pending(qc=qc, ht=ht, cps0=cps0, cps1=cps1):
                            def run():
                                emit_norm(qc, ht, cps0, cps1)
                            return run

                        pending = make_pending()
                        if ht == 1:
                            def make_out(qc=qc):
                                def run():
                                    emit_outproj(qc)
                                return run
                            pending_out = make_out()
                pending()
                pending_out()

    _split_excess_waits(nc)
    return nc


def _get_nc():
    if "nc" not in _state:
        _state["nc"] = _build_nc()
    return _state["nc"]


def _host_masks01():
    # mask01_j[k, q] = 1 if q >= 128*j + k else 0   (within a 512-q chunk)
    k = np.arange(128)[:, None]
    q = np.arange(512)[None, :]
    masks = np.empty((4, 128, 512), np.float32)
    for j in range(4):
        masks[j] = (q >= 128 * j + k).astype(np.float32)
    return masks


def _build_in_maps(x, Wq, bq, Wk, bk, Wv, bv, Wo):
    import ml_dtypes

    md = ml_dtypes.bfloat16

    x = np.asarray(x, np.float32)
    Wq = np.asarray(Wq, np.float32)
    bq = np.asarray(bq, np.float32)
    Wk = np.asarray(Wk, np.float32)
    bk = np.asarray(bk, np.float32)
    Wv = np.asarray(Wv, np.float32)
    bv = np.asarray(bv, np.float32)
    Wo = np.asarray(Wo, np.float32)

    masks = _host_masks01().astype(md)

    in_maps = []
    for core in range(NCORES):
        b, g = core // HG, core % HG
        cs = slice(g * DG, (g + 1) * DG)
        xT = np.ascontiguousarray(x[b].T).reshape(8, 128, S).astype(md)
        in_maps.append(
            {
                "xT": xT,
                "wq": np.ascontiguousarray(0.125 * Wq[:, cs]).reshape(8, 128, DG).astype(md),
                "wk": np.ascontiguousarray(Wk[:, cs]).reshape(8, 128, DG).astype(md),
                "wv": np.ascontiguousarray(Wv[:, cs]).reshape(8, 128, DG).astype(md),
                "wo": np.ascontiguousarray(Wo[cs, :]).reshape(2, 128, D_OUT).astype(md),
                "bq": (0.125 * bq[cs]).reshape(2, 128, 1).astype(np.float32),
                "bk": bk[cs].reshape(2, 128, 1).astype(np.float32),
                "bv": bv[cs].reshape(2, 128, 1).astype(np.float32),
                "masks": masks,
            }
        )
    return in_maps


def kernel(x, Wq, bq, Wk, bk, Wv, bv, Wo):
    from concourse.bass_utils import run_bass_kernel_spmd

    nc = _get_nc()
    in_maps = _build_in_maps(x, Wq, bq, Wk, bk, Wv, bv, Wo)
    _state["in_maps"] = in_maps

    res = run_bass_kernel_spmd(nc, in_maps, list(range(NCORES)))
    out = np.zeros((B, S, D_OUT), np.float64)
    for core in range(NCORES):
        out[core // HG] += np.asarray(res.results[core]["out"], np.float32)
    return out.astype(np.float32)


# revision 41
# speedup vs baseline: 1.1595x; 1.1595x over previous
"""Multi-head causal attention (B=2, S=2048, D=1024, H=16, Dh=64) on 8
axon-tunneled TRN2 NeuronCores.

Sharding: core = b*4 + g handles batch b and head group g (4 heads, 256
feature columns of the QKV projections / 256 rows of Wo).  Each core is
fully independent; the host sums the 4 per-head-group partial outputs of
each batch.

Per-core layout ("feature on partitions, seq on free"):
  xT   (1024, 2048)  = x[b].T                       (bf16)
  QT   (256, 2048)   = (0.125*Wq_g).T @ x.T + 0.125*bq_g   (scale in Wq)
  KT   (256, 2048)   = Wk_g.T @ x.T + bk_g
  va   (2048, 386)   = per even head [V_h | 1]; per odd head
                       [0*32 | 1 | 0*31 | V_h]  (so the PV matmul puts the
                       odd head's ctx at partitions 64:128 and its softmax
                       denominator at partition 32 - no partition-shifting
                       DMA needed afterwards)
  S^T tiles (128k, 512q) = KT_h[:, kblk].T @ QT_h[:, qchunk]  (contract 64)
  P^T  = exp(S^T) * mask01          (binary causal mask applied post-exp;
                                     no max-subtraction: |S| small)
  ctx_aug^T = sum_k va_h[kblk].T @ P^T              (PSUM accumulate)
  ctxT (256, 2048) = ctx_aug * (1/denom) + bv_h
  out_partial (2048, 1024) = ctxT.T @ Wo_g          (bf16, host sums in fp32)

Attention is software-pipelined: the two heads of a pair issue their
score matmuls back-to-back into different PE row groups (they run
concurrently - contraction is only 64), and the PV matmuls lag the score
matmuls by one k-block so the PE never sits behind the exp on the
scalar engine.  Normalization + output projection of a finished group
are emitted one group later, hiding the reciprocal-chain latency.
"""

import numpy as np

D_IN = 1024
D_OUT = 1024
H = 16
DH = 64
B = 2
S = 2048
NCORES = 8
HG = 4            # heads per core
DG = HG * DH      # 256 feature cols per core

MM_DT_NAME = "bfloat16"

_state = {}


def _patch_tile_drain():
    """This image's walrus rejects instructions carrying >2 sync waits
    ("Too many sync wait commands"); Tile's final drain waits on every
    outstanding proc.  Split the waits into single-wait SP nops."""
    import concourse.tile as tile
    from concourse import mybir
    from concourse.vector_clock import ScopedClock

    if getattr(tile.TileContext._drain_and_barrier, "_split_waits", False):
        return

    def _drain_and_barrier(self, tick_clock, wait_clock):
        nc = self.nc
        probe = nc.sync.nop()
        wait_clock.add_sem_waits(
            probe.ins, ScopedClock({None: tick_clock.global_clock})
        )
        si = probe.ins.sync_info
        waits = list(si.on_wait) if si and si.on_wait else []
        if len(waits) > 1:
            probe.ins.sync_info = mybir.SyncInfo(
                on_wait=[waits[0]], on_update=list(si.on_update or [])
            )
            for w in waits[1:]:
                extra = nc.sync.nop()
                extra.ins.sync_info = mybir.SyncInfo(on_wait=[w], on_update=[])
        nc.sync.drain()

        nc.all_engine_barrier()
        assert self.sems is not None
        popped = nc._tile_sem_poison_stack.pop()
        assert popped is self._sem_poison
        nc.clear_and_free_semaphores(list(self.sems.allocated().values()))
        nc.all_engine_barrier()

    _drain_and_barrier._split_waits = True
    tile.TileContext._drain_and_barrier = _drain_and_barrier


def _split_excess_waits(nc, maxw=1):
    """Walrus in this image rejects instructions with too many sync-wait
    commands.  Hoist excess waits onto InstNoOp carriers inserted right
    before the offending instruction on the same engine (engines are
    in-order, so this preserves semantics)."""
    from concourse import mybir

    f = nc.m.functions[0]
    for bb in f.blocks:
        insts = bb.instructions  # live list
        i = 0
        while i < len(insts):
            ins = insts[i]
            si = ins.sync_info
            waits = list(si.on_wait) if si and si.on_wait else []
            if len(waits) > maxw:
                excess, keep = waits[:-maxw], waits[-maxw:]
                nops = []
                for j in range(0, len(excess), maxw):
                    nop = mybir.InstNoOp(
                        name=f"I-waitnop-{nc.next_id()}", ins=[], outs=[]
                    )
                    nop.engine = ins.engine
                    nop.sync_info = mybir.SyncInfo(
                        on_wait=excess[j : j + maxw], on_update=[]
                    )
                    nops.append(nop)
                ins.sync_info = mybir.SyncInfo(
                    on_wait=keep, on_update=list(si.on_update or [])
                )
                insts[i:i] = nops
                i += len(nops)
            i += 1


# va column layout: even heads [V|1] (65 cols), odd heads
# [1 | zeros*63 | V] (128 cols - the ones column at position 0 puts the
# odd head's softmax denominator at out partition 0, its ctx at 64:128).
# Offsets per head:
VA_OFF = [0, 65, 193, 258]
VA_COLS = 386


def _build_nc():
    import concourse.bass as bass
    import concourse.tile as tile
    from concourse import mybir

    _patch_tile_drain()
    FP = mybir.dt.float32
    R = mybir.dt.float32r
    Alu = mybir.AluOpType
    Act = mybir.ActivationFunctionType

    assert MM_DT_NAME == "bfloat16"
    MD = mybir.dt.bfloat16

    nc = bass.Bass("TRN2", target_bir_lowering=False, debug=False)
    d_xT = nc.dram_tensor("xT", [8, 128, S], MD, kind="ExternalInput").ap()
    d_wq = nc.dram_tensor("wq", [8, 128, DG], MD, kind="ExternalInput").ap()
    d_wk = nc.dram_tensor("wk", [8, 128, DG], MD, kind="ExternalInput").ap()
    d_wv = nc.dram_tensor("wv", [8, 128, DG], MD, kind="ExternalInput").ap()
    d_wo = nc.dram_tensor("wo", [2, 128, D_OUT], MD, kind="ExternalInput").ap()
    d_bq = nc.dram_tensor("bq", [2, 128, 1], FP, kind="ExternalInput").ap()
    d_bk = nc.dram_tensor("bk", [2, 128, 1], FP, kind="ExternalInput").ap()
    d_bv = nc.dram_tensor("bv", [2, 128, 1], FP, kind="ExternalInput").ap()
    d_mask = nc.dram_tensor("masks", [4, 128, 512], MD, kind="ExternalInput").ap()
    d_out = nc.dram_tensor("out", [S, D_OUT], MD, kind="ExternalOutput").ap()

    with tile.TileContext(nc) as tc:
        from contextlib import ExitStack

        with ExitStack() as ctx:
            const = ctx.enter_context(tc.tile_pool(name="const", bufs=1))
            qkv = ctx.enter_context(tc.tile_pool(name="qkv", bufs=1))

            wq_sb = [const.tile([128, DG], MD, tag=f"wq{i}", name=f"wq{i}") for i in range(8)]
            wk_sb = [const.tile([128, DG], MD, tag=f"wk{i}", name=f"wk{i}") for i in range(8)]
            wv_sb = [const.tile([128, DG], MD, tag=f"wv{i}", name=f"wv{i}") for i in range(8)]
            wo_sb = [const.tile([128, D_OUT], MD, tag=f"wo{i}", name=f"wo{i}") for i in range(2)]
            bq_sb = [const.tile([128, 1], FP, tag=f"bq{i}", name=f"bq{i}") for i in range(2)]
            bk_sb = [const.tile([128, 1], FP, tag=f"bk{i}", name=f"bk{i}") for i in range(2)]
            bv_sb = [const.tile([128, 1], FP, tag=f"bv{i}", name=f"bv{i}") for i in range(2)]
            mask_sb = [const.tile([128, 512], MD, tag=f"mask{j}", name=f"mask{j}") for j in range(4)]
            # broadcast-ones rows for the denominator matmuls: row 64 feeds
            # even heads (denom at partition 64), row 32 odd heads (partition
            # 32).  fp32r: full accuracy, 2-cycles/row matmul (memset cannot
            # target fp32r, so write through a fp32 bitcast view).
            ones_sb = const.tile([65, DH], R, tag="ones")
            nc.vector.memset(ones_sb[64:65, :].bitcast(FP), 1.0)
            # full-width ones row at partition 0 for the odd-head broadcast
            # (dst partitions 64:128 require a 128-col matmul: col-group-64
            # dst with a 32-row tile fails the s3d3 ISA dst-partition check)
            ones2_sb = const.tile([1, 128], R, tag="ones2")
            nc.vector.memset(ones2_sb[:, :].bitcast(FP), 1.0)

            qT = [qkv.tile([128, S], MD, tag=f"qT{i}", name=f"qT{i}") for i in range(2)]
            kT = [qkv.tile([128, S], MD, tag=f"kT{i}", name=f"kT{i}") for i in range(2)]
            va = [qkv.tile([128, VA_COLS], MD, tag=f"va{i}", name=f"va{i}") for i in range(16)]
            ctxT = [qkv.tile([128, S], MD, tag=f"ctxT{i}", name=f"ctxT{i}") for i in range(2)]

            # va ones columns + odd-head zero padding
            for st in range(16):
                for h in (1, 3):
                    off = VA_OFF[h]
                    nc.vector.memset(va[st][:, off : off + 64], 0.0)
                    nc.vector.memset(va[st][:, off : off + 1], 1.0)
                for h in (0, 2):
                    off = VA_OFF[h]
                    nc.vector.memset(va[st][:, off + 64 : off + 65], 1.0)

            # ---------------- phase 1: projections ----------------
            # xpool stays open through phase 2: V-projection groups for
            # s-tiles 4..15 are interleaved between attention groups (the
            # exp-paced attention loop leaves the PE ~40% idle)
            xpool = ctx.enter_context(tc.tile_pool(name="xp", bufs=1))
            if True:
                xsb = [xpool.tile([128, S], MD, tag=f"x{i}", name=f"x{i}") for i in range(8)]
                # PE warm-up: dummy matmuls on resident scratch bridge the
                # initial input-DMA wait so the HAM clock gate opens before
                # the first real matmul (cold PE runs at 1.2 instead of
                # 2.4 GHz for its first ~3.4us of activity)
                warm = const.tile([1, 512], MD, tag="warm")
                nc.vector.memset(warm[:, :], 1.0)
                # spread input DMA over the three DMA-capable queues so the
                # x tiles land as fast as the fabric allows
                qs3 = [nc.sync, nc.scalar, nc.gpsimd]
                for i in range(8):
                    qs3[i % 3].dma_start(xsb[i][:], d_xT[i])
                    qs3[(i + 1) % 3].dma_start(wq_sb[i][:], d_wq[i])
                    qs3[(i + 2) % 3].dma_start(wk_sb[i][:], d_wk[i])
                    qs3[(i + 1) % 3].dma_start(wv_sb[i][:], d_wv[i])
                for i in range(2):
                    nc.scalar.dma_start(bq_sb[i][:], d_bq[i])
                    nc.gpsimd.dma_start(bk_sb[i][:], d_bk[i])
                    nc.sync.dma_start(bv_sb[i][:], d_bv[i])
                    nc.gpsimd.dma_start(wo_sb[i][:], d_wo[i])
                for j in range(4):
                    nc.scalar.dma_start(mask_sb[j][:], d_mask[j])

                # Q/K projections, ci-outer so the accumulation matmuls
                # pipeline with the x-tile DMA arrival order
                with tc.tile_pool(name="qkp", bufs=1, space="PSUM") as qkp:
                    wps = qkp.tile([128, 512], FP, tag="pq0", name="warmps")
                    for r in range(24):
                        nc.tensor.matmul(
                            wps[:, :], warm[0:1, 0:128], warm[0:1, :],
                            start=True, stop=True,
                        )
                    for m in range(2):
                        ms = slice(m * 128, (m + 1) * 128)
                        psQ = [qkp.tile([128, 512], FP, tag=f"pq{nq}", name=f"pq{m}{nq}") for nq in range(4)]
                        psK = [qkp.tile([128, 512], FP, tag=f"pk{nq}", name=f"pk{m}{nq}") for nq in range(4)]
                        for ci in range(8):
                            for nq in range(4):
                                sq = slice(nq * 512, (nq + 1) * 512)
                                nc.tensor.matmul(
                                    psQ[nq][:], wq_sb[ci][:, ms], xsb[ci][:, sq],
                                    start=(ci == 0), stop=(ci == 7),
                                )
                            for nq in range(4):
                                sq = slice(nq * 512, (nq + 1) * 512)
                                nc.tensor.matmul(
                                    psK[nq][:], wk_sb[ci][:, ms], xsb[ci][:, sq],
                                    start=(ci == 0), stop=(ci == 7),
                                )
                        for nq in range(4):
                            sq = slice(nq * 512, (nq + 1) * 512)
                            nc.vector.tensor_scalar(
                                qT[m][:, sq], psQ[nq][:], bq_sb[m][:], None, Alu.add
                            )
                            nc.vector.tensor_scalar(
                                kT[m][:, sq], psK[nq][:], bk_sb[m][:], None, Alu.add
                            )

                # V (natural layout) -> va tiles
                with tc.tile_pool(name="vp", bufs=1, space="PSUM") as vp:
                    psV = [vp.tile([128, DG], FP, tag=f"pv{i}", name=f"pv{i}") for i in range(4)]
                    for st in range(16):
                        ps = psV[st % 4]
                        ss = slice(st * 128, (st + 1) * 128)
                        for ci in range(8):
                            nc.tensor.matmul(
                                ps[:], xsb[ci][:, ss], wv_sb[ci][:],
                                start=(ci == 0), stop=(ci == 7),
                            )
                        for h in range(HG):
                            dst0 = VA_OFF[h] + (0 if h % 2 == 0 else 64)
                            nc.vector.tensor_copy(
                                va[st][:, dst0 : dst0 + 64],
                                ps[:, h * 64 : (h + 1) * 64],
                            )

            # ------- phase 2+3: pipelined attention + output projection -----
            # PSUM budget (8 banks): sps ring 2 + cps 2 tags x 2 bufs = 4 +
            # outproj/broadcast shared ring 2.  cps double-buffering is what
            # lets group g+1's first PV matmuls run while group g's
            # normalization chain (Ln/Exp on ACT) is still in flight.
            with tc.tile_pool(name="pt", bufs=4) as ptp, tc.tile_pool(
                name="norm", bufs=2
            ) as normp, tc.tile_pool(name="osb", bufs=3) as osb, tc.tile_pool(
                name="spsum", bufs=2, space="PSUM"
            ) as sp, tc.tile_pool(
                name="cpsum", bufs=1, space="PSUM"
            ) as cp, tc.tile_pool(
                name="opsum", bufs=2, space="PSUM"
            ) as op:

                def emit_vgroup(st):
                    """V-projection for s-tile st -> va[st] (psum from the
                    shared outproj ring; PE work fills exp-paced idle)."""
                    ps = op.tile([128, DG], FP, tag="o", name=f"pv{st}")
                    ss = slice(st * 128, (st + 1) * 128)
                    for ci in range(8):
                        nc.tensor.matmul(
                            ps[:], xsb[ci][:, ss], wv_sb[ci][:],
                            start=(ci == 0), stop=(ci == 7),
                        )
                    for h in range(HG):
                        dst0 = VA_OFF[h] + (0 if h % 2 == 0 else 64)
                        nc.vector.tensor_copy(
                            va[st][:, dst0 : dst0 + 64],
                            ps[:, h * 64 : (h + 1) * 64],
                        )

                def emit_norm(qc, ht, cps0, cps1):
                    """Normalize both heads of pair (qc, ht) and write ctxT."""
                    qsl = slice(qc * 512, (qc + 1) * 512)
                    rec0 = normp.tile([65, 512], FP, tag="rec0", name="rec0")
                    rec = normp.tile([65, 512], R, tag="rec", name="rec")
                    # gather both denominators (even head: partition 64 of
                    # cps0, odd head: partition 0 of cps1) into one tile so a
                    # single Ln+Exp pass covers the pair (rows 1..63 are
                    # garbage - processed but never read)
                    dd = normp.tile([65, 512], FP, tag="dd", name="dd")
                    nc.vector.tensor_copy(dd[64:65, :], cps0[64:65, :])
                    nc.vector.tensor_copy(dd[0:1, :], cps1[0:1, :])
                    nc.scalar.activation(rec0[0:65, :], dd[0:65, :], Act.Ln)
                    nc.scalar.activation(
                        rec[0:65, :], rec0[0:65, :], Act.Exp, scale=-1.0
                    )
                    # odd-head broadcast first (writes all 128 partitions;
                    # only 64:128 are consumed), then the even-head 64-row
                    # broadcast overwrites partitions 0:64
                    bc = op.tile([128, 512], FP, tag="o", name="bc")
                    nc.tensor.matmul(
                        bc[:, :], ones2_sb[0:1, :], rec[0:1, :],
                        start=True, stop=True,
                    )
                    bcs = normp.tile([128, 512], FP, tag="bcs", name="bcs")
                    nc.vector.tensor_copy(bcs[64:128, :], bc[64:128, :])
                    nc.tensor.matmul(
                        bc[0:64, :], ones_sb[64:65, :], rec[64:65, :],
                        start=True, stop=True,
                    )
                    nc.vector.tensor_copy(bcs[0:64, :], bc[0:64, :])
                    tmp = normp.tile([128, 512], FP, tag="tmp", name="tmp")
                    nc.vector.tensor_mul(tmp[0:64, :], cps0[0:64, :], bcs[0:64, :])
                    nc.vector.tensor_mul(
                        tmp[64:128, :], cps1[64:128, :], bcs[64:128, :]
                    )
                    # bias add on DVE, both heads in one op (gpsimd tensor
                    # ops cost ~7.5us each - keep it off gpsimd)
                    nc.vector.tensor_scalar(
                        ctxT[ht][:, qsl], tmp[:, :], bv_sb[ht][:, :],
                        None, Alu.add,
                    )

                def emit_outproj(qc):
                    oq = [nc.gpsimd, nc.sync, nc.scalar]
                    for st in range(4 * qc, 4 * qc + 4):
                        ss = slice(st * 128, (st + 1) * 128)
                        ot = osb.tile([128, 1024], MD, tag="ot", name="ot")
                        for n in range(2):
                            ns = slice(n * 512, (n + 1) * 512)
                            ps = op.tile([128, 512], FP, tag="o", name="o")
                            for cb in range(2):
                                nc.tensor.matmul(
                                    ps[:], ctxT[cb][:, ss], wo_sb[cb][:, ns],
                                    start=(cb == 0), stop=(cb == 1),
                                )
                            nc.vector.tensor_copy(ot[:, ns], ps[:])
                        oq[st % 3].dma_start(d_out[ss, :], ot[:])

                pending = None      # deferred norm emitter (fires at kb==1)
                pending_out = None  # deferred outproj emitter (fires at kb==4)
                for qc in range(4):
                    qs0 = qc * 512
                    for ht in range(2):
                        nkb = 4 * qc + 4
                        cps0 = cp.tile([65, 512], FP, tag="c0", name="c0")
                        cps1 = cp.tile([128, 512], FP, tag="c1", name="c1")
                        prev = None  # (kb, lo, pt0, pt1)
                        for kb in range(nkb):
                            ks = slice(kb * 128, (kb + 1) * 128)
                            j = kb - 4 * qc
                            lo = 128 * j if j > 0 else 0
                            qsl = slice(qs0 + lo, qs0 + 512)
                            # score matmuls for both heads, back-to-back into
                            # the two banks of one [128,1024] PSUM tile:
                            # different PE row groups -> run concurrently
                            sps = sp.tile([128, 1024], FP, tag="s", name="s")
                            for hp in range(2):
                                hs = slice(hp * 64, hp * 64 + 64)
                                nc.tensor.matmul(
                                    sps[:, 512 * hp + lo : 512 * (hp + 1)],
                                    kT[ht][hs, ks], qT[ht][hs, qsl],
                                    start=True, stop=True,
                                )
                            if kb == 1 and pending is not None:
                                pending()
                                pending = None
                            if kb == 4 and pending_out is not None:
                                pending_out()
                                pending_out = None
                            pt = ptp.tile([128, 1024], MD, tag="pt", name="pt")
                            if j < 0:
                                # off-diagonal: both halves fully live -> one
                                # wide exp covers the pair
                                nc.scalar.activation(pt[:, :], sps[:, :], Act.Exp)
                            else:
                                # diagonal: exp each half's written region,
                                # then zero the causal triangle post-exp with
                                # a binary bf16 mask (2x DVE)
                                for hp in range(2):
                                    h0c = 512 * hp + lo
                                    h1c = 512 * (hp + 1)
                                    nc.scalar.activation(
                                        pt[:, h0c:h1c], sps[:, h0c:h1c], Act.Exp
                                    )
                                    nc.vector.tensor_mul(
                                        pt[:, h0c:h1c], pt[:, h0c:h1c],
                                        mask_sb[j][:, lo:],
                                    )
                            if prev is not None:
                                pkb, plo, ppt = prev
                                h0, h1 = 2 * ht, 2 * ht + 1
                                nc.tensor.matmul(
                                    cps0[:, plo:],
                                    va[pkb][:, VA_OFF[h0] : VA_OFF[h0] + 65],
                                    ppt[:, plo:512],
                                    start=(pkb == 0), stop=(pkb == nkb - 1),
                                )
                                nc.tensor.matmul(
                                    cps1[:, plo:],
                                    va[pkb][:, VA_OFF[h1] : VA_OFF[h1] + 128],
                                    ppt[:, 512 + plo : 1024],
                                    start=(pkb == 0), stop=(pkb == nkb - 1),
                                )
                            prev = (kb, lo, pt)
                        # drain the last block's PV pair
                        pkb, plo, ppt = prev
                        h0, h1 = 2 * ht, 2 * ht + 1
                        nc.tensor.matmul(
                            cps0[:, plo:],
                            va[pkb][:, VA_OFF[h0] : VA_OFF[h0] + 65],
                            ppt[:, plo:512],
                            start=(pkb == 0), stop=(pkb == nkb - 1),
                        )
                        nc.tensor.matmul(
                            cps1[:, plo:],
                            va[pkb][:, VA_OFF[h1] : VA_OFF[h1] + 128],
                            ppt[:, 512 + plo : 1024],
                            start=(pkb == 0), stop=(pkb == nkb - 1),
                        )

                        def make_pending(qc=qc, ht=ht, cps0=cps0, cps1=cps1):
                            def run():
                                emit_norm(qc, ht, cps0, cps1)
                            return run

                        pending = make_pending()
                        if ht == 1:
                            def make_out(qc=qc):
                                def run():
                                    emit_outproj(qc)
                                return run
                            pending_out = make_out()
                pending()
                pending_out()

    _split_excess_waits(nc)
    return nc


def _get_nc():
    if "nc" not in _state:
        _state["nc"] = _build_nc()
    return _state["nc"]


def _host_masks01():
    # mask01_j[k, q] = 1 if q >= 128*j + k else 0   (within a 512-q chunk)
    k = np.arange(128)[:, None]
    q = np.arange(512)[None, :]
    masks = np.empty((4, 128, 512), np.float32)
    for j in range(4):
        masks[j] = (q >= 128 * j + k).astype(np.float32)
    return masks


def _build_in_maps(x, Wq, bq, Wk, bk, Wv, bv, Wo):
    import ml_dtypes

    md = ml_dtypes.bfloat16

    x = np.asarray(x, np.float32)
    Wq = np.asarray(Wq, np.float32)
    bq = np.asarray(bq, np.float32)
    Wk = np.asarray(Wk, np.float32)
    bk = np.asarray(bk, np.float32)
    Wv = np.asarray(Wv, np.float32)
    bv = np.asarray(bv, np.float32)
    Wo = np.asarray(Wo, np.float32)

    masks = _host_masks01().astype(md)

    in_maps = []
    for core in range(NCORES):
        b, g = core // HG, core % HG
        cs = slice(g * DG, (g + 1) * DG)
        xT = np.ascontiguousarray(x[b].T).reshape(8, 128, S).astype(md)
        in_maps.append(
            {
                "xT": xT,
                "wq": np.ascontiguousarray(0.125 * Wq[:, cs]).reshape(8, 128, DG).astype(md),
                "wk": np.ascontiguousarray(Wk[:, cs]).reshape(8, 128, DG).astype(md),
                "wv": np.ascontiguousarray(Wv[:, cs]).reshape(8, 128, DG).astype(md),
                "wo": np.ascontiguousarray(Wo[cs, :]).reshape(2, 128, D_OUT).astype(md),
                "bq": (0.125 * bq[cs]).reshape(2, 128, 1).astype(np.float32),
                "bk": bk[cs].reshape(2, 128, 1).astype(np.float32),
                "bv": bv[cs].reshape(2, 128, 1).astype(np.float32),
                "masks": masks,
            }
        )
    return in_maps


def kernel(x, Wq, bq, Wk, bk, Wv, bv, Wo):
    from concourse.bass_utils import run_bass_kernel_spmd

    nc = _get_nc()
    in_maps = _build_in_maps(x, Wq, bq, Wk, bk, Wv, bv, Wo)
    _state["in_maps"] = in_maps

    res = run_bass_kernel_spmd(nc, in_maps, list(range(NCORES)))
    out = np.zeros((B, S, D_OUT), np.float64)
    for core in range(NCORES):
        out[core // HG] += np.asarray(res.results[core]["out"], np.float32)
    return out.astype(np.float32)


# revision 43
# speedup vs baseline: 1.1854x; 1.0223x over previous
"""Multi-head causal attention (B=2, S=2048, D=1024, H=16, Dh=64) on 8
axon-tunneled TRN2 NeuronCores.

Sharding: core = b*4 + g handles batch b and head group g (4 heads, 256
feature columns of the QKV projections / 256 rows of Wo).  Each core is
fully independent; the host sums the 4 per-head-group partial outputs of
each batch.

Per-core layout ("feature on partitions, seq on free"):
  xT   (1024, 2048)  = x[b].T                       (bf16)
  QT   (256, 2048)   = (0.125*Wq_g).T @ x.T + 0.125*bq_g   (scale in Wq)
  KT   (256, 2048)   = Wk_g.T @ x.T + bk_g
  va   (2048, 386)   = per even head [V_h | 1]; per odd head
                       [0*32 | 1 | 0*31 | V_h]  (so the PV matmul puts the
                       odd head's ctx at partitions 64:128 and its softmax
                       denominator at partition 32 - no partition-shifting
                       DMA needed afterwards)
  S^T tiles (128k, 512q) = KT_h[:, kblk].T @ QT_h[:, qchunk]  (contract 64)
  P^T  = exp(S^T) * mask01          (binary causal mask applied post-exp;
                                     no max-subtraction: |S| small)
  ctx_aug^T = sum_k va_h[kblk].T @ P^T              (PSUM accumulate)
  ctxT (256, 2048) = ctx_aug * (1/denom) + bv_h
  out_partial (2048, 1024) = ctxT.T @ Wo_g          (bf16, host sums in fp32)

Attention is software-pipelined: the two heads of a pair issue their
score matmuls back-to-back into different PE row groups (they run
concurrently - contraction is only 64), and the PV matmuls lag the score
matmuls by one k-block so the PE never sits behind the exp on the
scalar engine.  Normalization + output projection of a finished group
are emitted one group later, hiding the reciprocal-chain latency.
"""

import numpy as np

D_IN = 1024
D_OUT = 1024
H = 16
DH = 64
B = 2
S = 2048
NCORES = 8
HG = 4            # heads per core
DG = HG * DH      # 256 feature cols per core

MM_DT_NAME = "bfloat16"

_state = {}


def _patch_tile_drain():
    """This image's walrus rejects instructions carrying >2 sync waits
    ("Too many sync wait commands"); Tile's final drain waits on every
    outstanding proc.  Split the waits into single-wait SP nops."""
    import concourse.tile as tile
    from concourse import mybir
    from concourse.vector_clock import ScopedClock

    if getattr(tile.TileContext._drain_and_barrier, "_split_waits", False):
        return

    def _drain_and_barrier(self, tick_clock, wait_clock):
        nc = self.nc
        probe = nc.sync.nop()
        wait_clock.add_sem_waits(
            probe.ins, ScopedClock({None: tick_clock.global_clock})
        )
        si = probe.ins.sync_info
        waits = list(si.on_wait) if si and si.on_wait else []
        if len(waits) > 1:
            probe.ins.sync_info = mybir.SyncInfo(
                on_wait=[waits[0]], on_update=list(si.on_update or [])
            )
            for w in waits[1:]:
                extra = nc.sync.nop()
                extra.ins.sync_info = mybir.SyncInfo(on_wait=[w], on_update=[])
        nc.sync.drain()

        nc.all_engine_barrier()
        assert self.sems is not None
        popped = nc._tile_sem_poison_stack.pop()
        assert popped is self._sem_poison
        nc.clear_and_free_semaphores(list(self.sems.allocated().values()))
        nc.all_engine_barrier()

    _drain_and_barrier._split_waits = True
    tile.TileContext._drain_and_barrier = _drain_and_barrier


def _split_excess_waits(nc, maxw=1):
    """Walrus in this image rejects instructions with too many sync-wait
    commands.  Hoist excess waits onto InstNoOp carriers inserted right
    before the offending instruction on the same engine (engines are
    in-order, so this preserves semantics)."""
    from concourse import mybir

    f = nc.m.functions[0]
    for bb in f.blocks:
        insts = bb.instructions  # live list
        i = 0
        while i < len(insts):
            ins = insts[i]
            si = ins.sync_info
            waits = list(si.on_wait) if si and si.on_wait else []
            if len(waits) > maxw:
                excess, keep = waits[:-maxw], waits[-maxw:]
                nops = []
                for j in range(0, len(excess), maxw):
                    nop = mybir.InstNoOp(
                        name=f"I-waitnop-{nc.next_id()}", ins=[], outs=[]
                    )
                    nop.engine = ins.engine
                    nop.sync_info = mybir.SyncInfo(
                        on_wait=excess[j : j + maxw], on_update=[]
                    )
                    nops.append(nop)
                ins.sync_info = mybir.SyncInfo(
                    on_wait=keep, on_update=list(si.on_update or [])
                )
                insts[i:i] = nops
                i += len(nops)
            i += 1


# va column layout: even heads [V|1] (65 cols), odd heads
# [1 | zeros*63 | V] (128 cols - the ones column at position 0 puts the
# odd head's softmax denominator at out partition 0, its ctx at 64:128).
# Offsets per head:
VA_OFF = [0, 65, 193, 258]
VA_COLS = 386


def _build_nc():
    import concourse.bass as bass
    import concourse.tile as tile
    from concourse import mybir

    _patch_tile_drain()
    FP = mybir.dt.float32
    R = mybir.dt.float32r
    Alu = mybir.AluOpType
    Act = mybir.ActivationFunctionType

    assert MM_DT_NAME == "bfloat16"
    MD = mybir.dt.bfloat16

    nc = bass.Bass("TRN2", target_bir_lowering=False, debug=False)
    d_xT = nc.dram_tensor("xT", [8, 128, S], MD, kind="ExternalInput").ap()
    d_wq = nc.dram_tensor("wq", [8, 128, DG], MD, kind="ExternalInput").ap()
    d_wk = nc.dram_tensor("wk", [8, 128, DG], MD, kind="ExternalInput").ap()
    d_wv = nc.dram_tensor("wv", [8, 128, DG], MD, kind="ExternalInput").ap()
    d_wo = nc.dram_tensor("wo", [2, 128, D_OUT], MD, kind="ExternalInput").ap()
    d_bq = nc.dram_tensor("bq", [2, 128, 1], FP, kind="ExternalInput").ap()
    d_bk = nc.dram_tensor("bk", [2, 128, 1], FP, kind="ExternalInput").ap()
    d_bv = nc.dram_tensor("bv", [2, 128, 1], FP, kind="ExternalInput").ap()
    d_mask = nc.dram_tensor("masks", [4, 128, 512], MD, kind="ExternalInput").ap()
    d_out = nc.dram_tensor("out", [S, D_OUT], MD, kind="ExternalOutput").ap()

    with tile.TileContext(nc) as tc:
        from contextlib import ExitStack

        with ExitStack() as ctx:
            const = ctx.enter_context(tc.tile_pool(name="const", bufs=1))
            qkv = ctx.enter_context(tc.tile_pool(name="qkv", bufs=1))

            wq_sb = [const.tile([128, DG], MD, tag=f"wq{i}", name=f"wq{i}") for i in range(8)]
            wk_sb = [const.tile([128, DG], MD, tag=f"wk{i}", name=f"wk{i}") for i in range(8)]
            wv_sb = [const.tile([128, DG], MD, tag=f"wv{i}", name=f"wv{i}") for i in range(8)]
            wo_sb = [const.tile([128, D_OUT], MD, tag=f"wo{i}", name=f"wo{i}") for i in range(2)]
            bq_sb = [const.tile([128, 1], FP, tag=f"bq{i}", name=f"bq{i}") for i in range(2)]
            bk_sb = [const.tile([128, 1], FP, tag=f"bk{i}", name=f"bk{i}") for i in range(2)]
            bv_sb = [const.tile([128, 1], FP, tag=f"bv{i}", name=f"bv{i}") for i in range(2)]
            mask_sb = [const.tile([128, 512], MD, tag=f"mask{j}", name=f"mask{j}") for j in range(4)]
            # broadcast-ones rows for the denominator matmuls: row 64 feeds
            # even heads (denom at partition 64), row 32 odd heads (partition
            # 32).  fp32r: full accuracy, 2-cycles/row matmul (memset cannot
            # target fp32r, so write through a fp32 bitcast view).
            ones_sb = const.tile([65, DH], R, tag="ones")
            nc.vector.memset(ones_sb[64:65, :].bitcast(FP), 1.0)
            # full-width ones row at partition 0 for the odd-head broadcast
            # (dst partitions 64:128 require a 128-col matmul: col-group-64
            # dst with a 32-row tile fails the s3d3 ISA dst-partition check)
            ones2_sb = const.tile([1, 128], R, tag="ones2")
            nc.vector.memset(ones2_sb[:, :].bitcast(FP), 1.0)

            qT = [qkv.tile([128, S], MD, tag=f"qT{i}", name=f"qT{i}") for i in range(2)]
            kT = [qkv.tile([128, S], MD, tag=f"kT{i}", name=f"kT{i}") for i in range(2)]
            va = [qkv.tile([128, VA_COLS], MD, tag=f"va{i}", name=f"va{i}") for i in range(16)]
            ctxT = [qkv.tile([128, S], MD, tag=f"ctxT{i}", name=f"ctxT{i}") for i in range(2)]

            # va ones columns + odd-head zero padding
            for st in range(16):
                for h in (1, 3):
                    off = VA_OFF[h]
                    nc.vector.memset(va[st][:, off : off + 64], 0.0)
                    nc.vector.memset(va[st][:, off : off + 1], 1.0)
                for h in (0, 2):
                    off = VA_OFF[h]
                    nc.vector.memset(va[st][:, off + 64 : off + 65], 1.0)

            # ---------------- phase 1: projections ----------------
            # xpool stays open through phase 2: V-projection groups for
            # s-tiles 4..15 are interleaved between attention groups (the
            # exp-paced attention loop leaves the PE ~40% idle)
            xpool = ctx.enter_context(tc.tile_pool(name="xp", bufs=1))
            if True:
                xsb = [xpool.tile([128, S], MD, tag=f"x{i}", name=f"x{i}") for i in range(8)]
                # spread input DMA over the three DMA-capable queues so the
                # x tiles land as fast as the fabric allows
                qs3 = [nc.sync, nc.scalar, nc.gpsimd]
                for i in range(8):
                    qs3[i % 3].dma_start(xsb[i][:], d_xT[i])
                    qs3[(i + 1) % 3].dma_start(wq_sb[i][:], d_wq[i])
                    qs3[(i + 2) % 3].dma_start(wk_sb[i][:], d_wk[i])
                    qs3[(i + 1) % 3].dma_start(wv_sb[i][:], d_wv[i])
                for i in range(2):
                    nc.scalar.dma_start(bq_sb[i][:], d_bq[i])
                    nc.gpsimd.dma_start(bk_sb[i][:], d_bk[i])
                    nc.sync.dma_start(bv_sb[i][:], d_bv[i])
                    nc.gpsimd.dma_start(wo_sb[i][:], d_wo[i])
                for j in range(4):
                    nc.scalar.dma_start(mask_sb[j][:], d_mask[j])

                # Q/K projections, ci-outer so the accumulation matmuls
                # pipeline with the x-tile DMA arrival order
                with tc.tile_pool(name="qkp", bufs=1, space="PSUM") as qkp:
                    for m in range(2):
                        ms = slice(m * 128, (m + 1) * 128)
                        psQ = [qkp.tile([128, 512], FP, tag=f"pq{nq}", name=f"pq{m}{nq}") for nq in range(4)]
                        psK = [qkp.tile([128, 512], FP, tag=f"pk{nq}", name=f"pk{m}{nq}") for nq in range(4)]
                        for ci in range(8):
                            for nq in range(4):
                                sq = slice(nq * 512, (nq + 1) * 512)
                                nc.tensor.matmul(
                                    psQ[nq][:], wq_sb[ci][:, ms], xsb[ci][:, sq],
                                    start=(ci == 0), stop=(ci == 7),
                                )
                            for nq in range(4):
                                sq = slice(nq * 512, (nq + 1) * 512)
                                nc.tensor.matmul(
                                    psK[nq][:], wk_sb[ci][:, ms], xsb[ci][:, sq],
                                    start=(ci == 0), stop=(ci == 7),
                                )
                        for nq in range(4):
                            sq = slice(nq * 512, (nq + 1) * 512)
                            nc.vector.tensor_scalar(
                                qT[m][:, sq], psQ[nq][:], bq_sb[m][:], None, Alu.add
                            )
                            nc.vector.tensor_scalar(
                                kT[m][:, sq], psK[nq][:], bk_sb[m][:], None, Alu.add
                            )

                # V (natural layout) -> va tiles
                with tc.tile_pool(name="vp", bufs=1, space="PSUM") as vp:
                    psV = [vp.tile([128, DG], FP, tag=f"pv{i}", name=f"pv{i}") for i in range(4)]
                    for st in range(16):
                        ps = psV[st % 4]
                        ss = slice(st * 128, (st + 1) * 128)
                        for ci in range(8):
                            nc.tensor.matmul(
                                ps[:], xsb[ci][:, ss], wv_sb[ci][:],
                                start=(ci == 0), stop=(ci == 7),
                            )
                        for h in range(HG):
                            dst0 = VA_OFF[h] + (0 if h % 2 == 0 else 64)
                            nc.vector.tensor_copy(
                                va[st][:, dst0 : dst0 + 64],
                                ps[:, h * 64 : (h + 1) * 64],
                            )

            # ------- phase 2+3: pipelined attention + output projection -----
            # PSUM budget (8 banks): sps ring 2 + cps 2 tags x 2 bufs = 4 +
            # outproj/broadcast shared ring 2.  cps double-buffering is what
            # lets group g+1's first PV matmuls run while group g's
            # normalization chain (Ln/Exp on ACT) is still in flight.
            with tc.tile_pool(name="pt", bufs=4) as ptp, tc.tile_pool(
                name="norm", bufs=2
            ) as normp, tc.tile_pool(name="osb", bufs=3) as osb, tc.tile_pool(
                name="spsum", bufs=2, space="PSUM"
            ) as sp, tc.tile_pool(
                name="cpsum", bufs=1, space="PSUM"
            ) as cp, tc.tile_pool(
                name="opsum", bufs=2, space="PSUM"
            ) as op:

                def emit_vgroup(st):
                    """V-projection for s-tile st -> va[st] (psum from the
                    shared outproj ring; PE work fills exp-paced idle)."""
                    ps = op.tile([128, DG], FP, tag="o", name=f"pv{st}")
                    ss = slice(st * 128, (st + 1) * 128)
                    for ci in range(8):
                        nc.tensor.matmul(
                            ps[:], xsb[ci][:, ss], wv_sb[ci][:],
                            start=(ci == 0), stop=(ci == 7),
                        )
                    for h in range(HG):
                        dst0 = VA_OFF[h] + (0 if h % 2 == 0 else 64)
                        nc.vector.tensor_copy(
                            va[st][:, dst0 : dst0 + 64],
                            ps[:, h * 64 : (h + 1) * 64],
                        )

                def emit_norm(qc, ht, cps0, cps1):
                    """Normalize both heads of pair (qc, ht) and write ctxT."""
                    qsl = slice(qc * 512, (qc + 1) * 512)
                    rec0 = normp.tile([65, 512], FP, tag="rec0", name="rec0")
                    rec = normp.tile([65, 512], R, tag="rec", name="rec")
                    # gather both denominators (even head: partition 64 of
                    # cps0, odd head: partition 0 of cps1) into one tile so a
                    # single Ln+Exp pass covers the pair (rows 1..63 are
                    # garbage - processed but never read)
                    dd = normp.tile([65, 512], FP, tag="dd", name="dd")
                    nc.vector.tensor_copy(dd[64:65, :], cps0[64:65, :])
                    nc.vector.tensor_copy(dd[0:1, :], cps1[0:1, :])
                    nc.scalar.activation(rec0[0:65, :], dd[0:65, :], Act.Ln)
                    nc.scalar.activation(
                        rec[0:65, :], rec0[0:65, :], Act.Exp, scale=-1.0
                    )
                    # odd-head broadcast first (writes all 128 partitions;
                    # only 64:128 are consumed), then the even-head 64-row
                    # broadcast overwrites partitions 0:64
                    bc = op.tile([128, 512], FP, tag="o", name="bc")
                    nc.tensor.matmul(
                        bc[:, :], ones2_sb[0:1, :], rec[0:1, :],
                        start=True, stop=True,
                    )
                    bcs = normp.tile([128, 512], FP, tag="bcs", name="bcs")
                    nc.vector.tensor_copy(bcs[64:128, :], bc[64:128, :])
                    nc.tensor.matmul(
                        bc[0:64, :], ones_sb[64:65, :], rec[64:65, :],
                        start=True, stop=True,
                    )
                    nc.vector.tensor_copy(bcs[0:64, :], bc[0:64, :])
                    tmp = normp.tile([128, 512], FP, tag="tmp", name="tmp")
                    nc.vector.tensor_mul(tmp[0:64, :], cps0[0:64, :], bcs[0:64, :])
                    nc.vector.tensor_mul(
                        tmp[64:128, :], cps1[64:128, :], bcs[64:128, :]
                    )
                    # bias add on DVE, both heads in one op (gpsimd tensor
                    # ops cost ~7.5us each - keep it off gpsimd)
                    nc.vector.tensor_scalar(
                        ctxT[ht][:, qsl], tmp[:, :], bv_sb[ht][:, :],
                        None, Alu.add,
                    )

                def emit_outproj(qc):
                    oq = [nc.gpsimd, nc.sync, nc.scalar]
                    for st in range(4 * qc, 4 * qc + 4):
                        ss = slice(st * 128, (st + 1) * 128)
                        ot = osb.tile([128, 1024], MD, tag="ot", name="ot")
                        for n in range(2):
                            ns = slice(n * 512, (n + 1) * 512)
                            ps = op.tile([128, 512], FP, tag="o", name="o")
                            for cb in range(2):
                                nc.tensor.matmul(
                                    ps[:], ctxT[cb][:, ss], wo_sb[cb][:, ns],
                                    start=(cb == 0), stop=(cb == 1),
                                )
                            nc.vector.tensor_copy(ot[:, ns], ps[:])
                        oq[st % 3].dma_start(d_out[ss, :], ot[:])

                pending = None      # deferred norm emitter (fires at kb==1)
                pending_out = None  # deferred outproj emitter (fires at kb==4)
                for qc in range(4):
                    qs0 = qc * 512
                    for ht in range(2):
                        nkb = 4 * qc + 4
                        cps0 = cp.tile([65, 512], FP, tag="c0", name="c0")
                        cps1 = cp.tile([128, 512], FP, tag="c1", name="c1")
                        prev = None  # (kb, lo, pt0, pt1)
                        for kb in range(nkb):
                            ks = slice(kb * 128, (kb + 1) * 128)
                            j = kb - 4 * qc
                            lo = 128 * j if j > 0 else 0
                            qsl = slice(qs0 + lo, qs0 + 512)
                            # score matmuls for both heads, back-to-back into
                            # the two banks of one [128,1024] PSUM tile:
                            # different PE row groups -> run concurrently
                            sps = sp.tile([128, 1024], FP, tag="s", name="s")
                            for hp in range(2):
                                hs = slice(hp * 64, hp * 64 + 64)
                                nc.tensor.matmul(
                                    sps[:, 512 * hp + lo : 512 * (hp + 1)],
                                    kT[ht][hs, ks], qT[ht][hs, qsl],
                                    start=True, stop=True,
                                )
                            if kb == 1 and pending is not None:
                                pending()
                                pending = None
                            if kb == 4 and pending_out is not None:
                                pending_out()
                                pending_out = None
                            pt = ptp.tile([128, 1024], MD, tag="pt", name="pt")
                            if j < 0:
                                # off-diagonal: both halves fully live -> one
                                # wide exp covers the pair
                                nc.scalar.activation(pt[:, :], sps[:, :], Act.Exp)
                            else:
                                # diagonal: exp each half's written region,
                                # then zero the causal triangle post-exp with
                                # a binary bf16 mask (2x DVE)
                                for hp in range(2):
                                    h0c = 512 * hp + lo
                                    h1c = 512 * (hp + 1)
                                    nc.scalar.activation(
                                        pt[:, h0c:h1c], sps[:, h0c:h1c], Act.Exp
                                    )
                                    nc.vector.tensor_mul(
                                        pt[:, h0c:h1c], pt[:, h0c:h1c],
                                        mask_sb[j][:, lo:],
                                    )
                            if prev is not None:
                                pkb, plo, ppt = prev
                                h0, h1 = 2 * ht, 2 * ht + 1
                                nc.tensor.matmul(
                                    cps0[:, plo:],
                                    va[pkb][:, VA_OFF[h0] : VA_OFF[h0] + 65],
                                    ppt[:, plo:512],
                                    start=(pkb == 0), stop=(pkb == nkb - 1),
                                )
                                nc.tensor.matmul(
                                    cps1[:, plo:],
                                    va[pkb][:, VA_OFF[h1] : VA_OFF[h1] + 128],
                                    ppt[:, 512 + plo : 1024],
                                    start=(pkb == 0), stop=(pkb == nkb - 1),
                                )
                            prev = (kb, lo, pt)
                        # drain the last block's PV pair
                        pkb, plo, ppt = prev
                        h0, h1 = 2 * ht, 2 * ht + 1
                        nc.tensor.matmul(
                            cps0[:, plo:],
                            va[pkb][:, VA_OFF[h0] : VA_OFF[h0] + 65],
                            ppt[:, plo:512],
                            start=(pkb == 0), stop=(pkb == nkb - 1),
                        )
                        nc.tensor.matmul(
                            cps1[:, plo:],
                            va[pkb][:, VA_OFF[h1] : VA_OFF[h1] + 128],
                            ppt[:, 512 + plo : 1024],
                            start=(pkb == 0), stop=(pkb == nkb - 1),
                        )

                        def make_pending(qc=qc, ht=ht, cps0=cps0, cps1=cps1):
                            def run():
                                emit_norm(qc, ht, cps0, cps1)
                            return run

                        pending = make_pending()
                        if ht == 1:
                            def make_out(qc=qc):
                                def run():
                                    emit_outproj(qc)
                                return run
                            pending_out = make_out()
                pending()
                pending_out()

    _split_excess_waits(nc)
    return nc


def _get_nc():
    if "nc" not in _state:
        _state["nc"] = _build_nc()
    return _state["nc"]


def _host_masks01():
    # mask01_j[k, q] = 1 if q >= 128*j + k else 0   (within a 512-q chunk)
    k = np.arange(128)[:, None]
    q = np.arange(512)[None, :]
    masks = np.empty((4, 128, 512), np.float32)
    for j in range(4):
        masks[j] = (q >= 128 * j + k).astype(np.float32)
    return masks


def _build_in_maps(x, Wq, bq, Wk, bk, Wv, bv, Wo):
    import ml_dtypes

    md = ml_dtypes.bfloat16

    x = np.asarray(x, np.float32)
    Wq = np.asarray(Wq, np.float32)
    bq = np.asarray(bq, np.float32)
    Wk = np.asarray(Wk, np.float32)
    bk = np.asarray(bk, np.float32)
    Wv = np.asarray(Wv, np.float32)
    bv = np.asarray(bv, np.float32)
    Wo = np.asarray(Wo, np.float32)

    masks = _host_masks01().astype(md)

    in_maps = []
    for core in range(NCORES):
        b, g = core // HG, core % HG
        cs = slice(g * DG, (g + 1) * DG)
        xT = np.ascontiguousarray(x[b].T).reshape(8, 128, S).astype(md)
        in_maps.append(
            {
                "xT": xT,
                "wq": np.ascontiguousarray(0.125 * Wq[:, cs]).reshape(8, 128, DG).astype(md),
                "wk": np.ascontiguousarray(Wk[:, cs]).reshape(8, 128, DG).astype(md),
                "wv": np.ascontiguousarray(Wv[:, cs]).reshape(8, 128, DG).astype(md),
                "wo": np.ascontiguousarray(Wo[cs, :]).reshape(2, 128, D_OUT).astype(md),
                "bq": (0.125 * bq[cs]).reshape(2, 128, 1).astype(np.float32),
                "bk": bk[cs].reshape(2, 128, 1).astype(np.float32),
                "bv": bv[cs].reshape(2, 128, 1).astype(np.float32),
                "masks": masks,
            }
        )
    return in_maps


def kernel(x, Wq, bq, Wk, bk, Wv, bv, Wo):
    from concourse.bass_utils import run_bass_kernel_spmd

    nc = _get_nc()
    in_maps = _build_in_maps(x, Wq, bq, Wk, bk, Wv, bv, Wo)
    _state["in_maps"] = in_maps

    res = run_bass_kernel_spmd(nc, in_maps, list(range(NCORES)))
    out = np.zeros((B, S, D_OUT), np.float64)
    for core in range(NCORES):
        out[core // HG] += np.asarray(res.results[core]["out"], np.float32)
    return out.astype(np.float32)
